# revision 13
# baseline (speedup 1.0000x reference)
"""Multi-head attention Trainium2 Bass kernel.

Problem: B=4, N=M=2048, DM=512, H=8, DH=64, DO=512, fp32.
Sharding: 8 cores = (batch b, row-half) -- each core computes full attention
for 1024 query rows of one batch. No collectives.

Per-core dataflow (v2 -- oh flipped to [n, 65], bf16 attention operands):
  - PE-transpose Q,K,V 128x128 blocks (bf16 identity -> 1 cyc/row);
    transposed K/Q staging persists so per-head projections can be
    interleaved into later attention windows.
  - kTf/qTf [hdh, m|n] bf16 (bias + 1/sqrt(dh) folded host-side)
  - vha [m, h, 65] bf16 = [Vh + vb | 1]  (v-bias exact since sum(attn)=1)
  - scoresT[m, n] = kh @ qhT per head pair (tile_position row packing)
  - exp on ScalarE (PSUM fp32 -> SBUF bf16)
  - oh[n, 65] = ex^T(stationary) @ vha(moving, F=65); col 64 = denominator
  - normalize on DVE: per-partition reciprocal + multiply -> mh2 bf16
  - PE-transpose mh2 -> mhT [hdh, n] bf16
  - out[n, do] = sum_hp mhT_hp^T @ wp_hp + bias (ones-row matmul), PSUM->HBM
Loop nest: hp (head pair) outer, nb (n-half) inner; window w = hp*2+nb.
oh of window w-1 (+normalize+transpose) interleaves into window w's
scores/exp; V projection fills window 0; kTf/qTf head-pair projections fill
windows 1-3; output projections of nb0 fill window 7; nb1 outputs tail.
"""
import os
import sys

sys.path.insert(0, "/opt/trn_rl_repo")

import numpy as np
import ml_dtypes

import concourse.bass as bass
import concourse.mybir as mybir
import concourse.tile as tile
from concourse import bacc
from concourse.bass_utils import run_bass_kernel_spmd

F32 = mybir.dt.float32
F32R = mybir.dt.float32r
BF16 = mybir.dt.bfloat16
EXP = mybir.ActivationFunctionType.Exp
ADD = mybir.AluOpType.add
MULT = mybir.AluOpType.mult

P = 128
DM = 512
HDH = 512
DH = 64
H = 8
NB = 1024     # query rows per core
M = 2048      # kv rows
DO = 512
N_MT = M // P
N_QT = NB // P

_CACHED = {}
LAST_EXEC_NS = None
_SECTION = None  # optional trace-attribution hook: list whose [0] is set


def _mark(s):
    if _SECTION is not None:
        _SECTION[0] = s


def _build():
    nc = bacc.Bacc("TRN2", target_bir_lowering=False, debug=False)

    d_q = nc.declare_dram_parameter("q", [NB, DM], F32, isOutput=False)
    d_k = nc.declare_dram_parameter("k", [M, DM], F32, isOutput=False)
    d_v = nc.declare_dram_parameter("v", [M, DM], F32, isOutput=False)
    d_wq = nc.declare_dram_parameter("wq", [DM, HDH], F32R, isOutput=False)
    d_wk = nc.declare_dram_parameter("wk", [DM, HDH], F32R, isOutput=False)
    d_wv = nc.declare_dram_parameter("wv", [DM, HDH], F32R, isOutput=False)
    d_wp = nc.declare_dram_parameter("wp", [HDH, DO], BF16, isOutput=False)
    d_qb = nc.declare_dram_parameter("qb", [P, 4], F32, isOutput=False)
    d_kb = nc.declare_dram_parameter("kb", [P, 4], F32, isOutput=False)
    d_vbrow = nc.declare_dram_parameter("vbrow", [1, HDH], F32R, isOutput=False)
    d_pb = nc.declare_dram_parameter("pb", [1, DO], F32R, isOutput=False)
    d_idb = nc.declare_dram_parameter("identb", [P, P], BF16, isOutput=False)
    d_id = nc.declare_dram_parameter("ident", [P, P], F32R, isOutput=False)
    d_ones = nc.declare_dram_parameter("ones", [P, P], F32R, isOutput=False)
    d_out = nc.declare_dram_parameter("out", [NB, DO], F32, isOutput=True)

    with tile.TileContext(nc) as tc:
        from contextlib import ExitStack
        with ExitStack() as ctx:
            persist = ctx.enter_context(tc.tile_pool(name="persist", bufs=1))
            raw = ctx.enter_context(tc.tile_pool(name="raw", bufs=2))
            vtt_pool = ctx.enter_context(tc.tile_pool(name="vtt", bufs=3))
            ex_pool = ctx.enter_context(tc.tile_pool(name="expp", bufs=19))
            nm = ctx.enter_context(tc.tile_pool(name="nm", bufs=4))
            mh2_pool = ctx.enter_context(tc.tile_pool(name="mh2", bufs=3))
            ps_sc = ctx.enter_context(tc.tile_pool(name="ps_sc", bufs=3, space="PSUM"))
            ps_wk = ctx.enter_context(tc.tile_pool(name="ps_wk", bufs=2, space="PSUM"))

            # --- constants (first DMAs out) ---
            identb = persist.tile([P, P], BF16, tag="identb", name="identb")
            nc.sync.dma_start(identb[:], d_idb[:])
            ident = persist.tile([P, P], F32R, tag="ident", name="ident")
            nc.sync.dma_start(ident[:], d_id[:])
            qb = persist.tile([P, 4], F32, tag="qb", name="qb")
            nc.sync.dma_start(qb[:], d_qb[:])
            kb = persist.tile([P, 4], F32, tag="kb", name="kb")
            nc.sync.dma_start(kb[:], d_kb[:])
            ones = persist.tile([P, P], F32R, tag="ones", name="ones")
            nc.sync.dma_start(ones[:], d_ones[:])

            # --- persistent tensors ---
            kTf = [persist.tile([P, M], BF16, tag=f"kTf{i}", name=f"kTf{i}")
                   for i in range(4)]
            qTf = [persist.tile([P, NB], BF16, tag=f"qTf{i}", name=f"qTf{i}")
                   for i in range(4)]
            ktsK = [persist.tile([P, 4, 512], F32R, tag=f"ktsK{i}", name=f"ktsK{i}")
                    for i in range(4)]
            ktsQ = [persist.tile([P, 4, 512], F32R, tag=f"ktsQ{i}", name=f"ktsQ{i}")
                    for i in range(2)]
            vha = persist.tile([P, N_MT, H, 65], BF16, tag="vha", name="vha")
            mhT = [[persist.tile([P, 512], BF16, tag=f"mhT{nb}_{hp}",
                                 name=f"mhT{nb}_{hp}")
                    for hp in range(4)] for nb in range(2)]
            vbb = persist.tile([P, H, DH], BF16, tag="vbb", name="vbb")
            pb = persist.tile([1, DO], F32R, tag="pb", name="pb")
            vbrow = persist.tile([1, HDH], F32R, tag="vbrow", name="vbrow")
            wk_sb = [persist.tile([P, HDH], F32R, tag=f"wk{d}", name=f"wk{d}")
                     for d in range(4)]
            wq_sb = [persist.tile([P, HDH], F32R, tag=f"wq{d}", name=f"wq{d}")
                     for d in range(4)]
            wv_sb = [persist.tile([P, HDH], F32R, tag=f"wv{d}", name=f"wv{d}")
                     for d in range(4)]
            wp_sb = persist.tile([P, 4, DO], BF16, tag="wp", name="wp")

            def load_group(d_src, t0, eng=None):
                """One DMA loading rows [t0*P, (t0+4)*P) as [p, j, c]."""
                stage = raw.tile([P, 4, DM], F32R, tag="araw", name="araw")
                (eng or nc.sync).dma_start(
                    stage[:],
                    d_src[t0 * P:(t0 + 4) * P, :].bitcast(F32R).rearrange(
                        "(j p) c -> p j c", p=P))
                return stage

            def transpose_tiles(stage, ts):
                """Transpose 4 row-tiles from stage into ts [P, 4, 512]
                ([dm-chunk, dc, row]). Copies on ScalarE (idle outside the
                attention windows)."""
                _mark("in_transpose")
                for j in range(4):
                    rn = stage[:, j, :]
                    pst = ps_wk.tile([P, DM], F32R, tag="pj", name="pj")
                    for dc in range(4):
                        nc.tensor.transpose(
                            pst[:, dc * P:(dc + 1) * P], rn[:, dc * P:(dc + 1) * P],
                            ident[:],
                        )
                    nc.scalar.copy(
                        ts[:, :, j * P:(j + 1) * P],
                        pst.rearrange("p (a b) -> p a b", a=4),
                    )

            def proj_k(ht, ms):
                """kTf[ht][:, ms*512:(ms+1)*512] from ktsK[ms]."""
                _mark(f"proj_k")
                pp = ps_sc.tile([P, 1024], F32, tag="sc", name="sc")
                for dc in range(4):
                    nc.tensor.matmul(
                        pp[:, 0:512], wk_sb[dc][:, ht * P:(ht + 1) * P],
                        ktsK[ms][:, dc, :], start=(dc == 0), stop=(dc == 3),
                    )
                nc.vector.tensor_scalar(
                    kTf[ht][:, ms * 512:(ms + 1) * 512],
                    pp[:, 0:512], kb[:, ht:ht + 1], None, ADD,
                )

            def proj_q(ht, ns):
                _mark(f"proj_q")
                pp = ps_sc.tile([P, 1024], F32, tag="sc", name="sc")
                for dc in range(4):
                    nc.tensor.matmul(
                        pp[:, 0:512], wq_sb[dc][:, ht * P:(ht + 1) * P],
                        ktsQ[ns][:, dc, :], start=(dc == 0), stop=(dc == 3),
                    )
                nc.vector.tensor_scalar(
                    qTf[ht][:, ns * 512:(ns + 1) * 512],
                    pp[:, 0:512], qb[:, ht:ht + 1], None, ADD,
                )

            # === lead-in: K transposes + kTf[0]; Q transposes + qTf[0] ===
            stage_k0 = load_group(d_k, 0)
            for dcc in range(4):
                nc.sync.dma_start(wk_sb[dcc][:], d_wk[dcc * P:(dcc + 1) * P, :])
            stages = {0: stage_k0}
            for ms in range(1, 4):
                stages[ms] = load_group(d_k, ms * 4,
                                        eng=nc.gpsimd if ms >= 2 else nc.sync)
            for ms in range(4):
                transpose_tiles(stages[ms], ktsK[ms])
                proj_k(0, ms)
            for dcc in range(4):
                nc.gpsimd.dma_start(wq_sb[dcc][:], d_wq[dcc * P:(dcc + 1) * P, :])
            stq = [load_group(d_q, 0), load_group(d_q, 4, eng=nc.gpsimd)]
            for ns in range(2):
                transpose_tiles(stq[ns], ktsQ[ns])
                proj_q(0, ns)
            for dcc in range(4):
                nc.gpsimd.dma_start(wv_sb[dcc][:], d_wv[dcc * P:(dcc + 1) * P, :])
            nc.gpsimd.dma_start(vbrow[:], d_vbrow[:])
            nc.gpsimd.dma_start(pb[:], d_pb[:])
            for a in range(4):
                nc.gpsimd.dma_start(wp_sb[:, a, :], d_wp[a * P:(a + 1) * P, :])
            # vbb = ones-col x vbrow: v-bias broadcast over m partitions
            bb = ps_wk.tile([P, DM], F32, tag="pj", name="pj")
            nc.tensor.matmul(bb[:], ones[0:1, 0:P], vbrow[:],
                             start=True, stop=True)
            nc.vector.tensor_copy(vbb.rearrange("p a b -> p (a b)"), bb[:])
            # ones column of vha
            nc.vector.tensor_copy(
                vha[:, :, :, 64:65],
                ones[:, 0:N_MT * H].bitcast(F32).rearrange(
                    "p (a h) -> p a h", a=N_MT)[:, :, :, None],
            )

            _vstages = {}

            def emit_v_tile(mt):
                """Transpose + project one V m-tile into vha (+v-bias)."""
                _mark("v_tile")
                g, j = mt // 4, mt % 4
                if g not in _vstages:
                    _vstages[g] = load_group(d_v, g * 4,
                                             eng=nc.gpsimd if g % 2 else nc.sync)
                vn = _vstages[g][:, j, :]
                pst = ps_wk.tile([P, DM], F32R, tag="pj", name="pj")
                for dc in range(4):
                    nc.tensor.transpose(
                        pst[:, dc * P:(dc + 1) * P], vn[:, dc * P:(dc + 1) * P],
                        ident[:],
                    )
                vtt = vtt_pool.tile([P, 4, P], F32R, tag="vtt", name="vtt")
                nc.vector.tensor_copy(vtt[:], pst.rearrange("p (a b) -> p a b", a=4))
                pp = ps_wk.tile([P, DM], F32, tag="pj", name="pj")
                for dc in range(4):
                    nc.tensor.matmul(
                        pp[:], vtt[:, dc, :], wv_sb[dc][:],
                        start=(dc == 0), stop=(dc == 3),
                    )
                nc.vector.tensor_tensor(
                    vha[:, mt, :, 0:64],
                    pp.rearrange("p (h c) -> p h c", h=H), vbb[:], ADD,
                )

            # recip consts
            from concourse.dve_ops import (
                RECIP_APPROX_FAST_CONSTS, RECIPROCAL_APPROX_FAST)
            _rc = RECIP_APPROX_FAST_CONSTS
            _mh2 = {}

            def oh_group(w, g, ex_tiles):
                """One oh accumulation group of window w: g = ab*4 + j.
                Accumulates oh[n-block j, 65] over all 16 m-tiles, then
                normalizes into mh2; emits the mh transpose after ab==1."""
                hp, nb = w // 2, w % 2
                ab, j = g // 4, g % 4
                _mark(f"oh_w{w}")
                h = 2 * hp + ab
                oh = ps_wk.tile([P, 512], F32, tag="pj", name="pj")
                for mu in range(8):
                    for jj in range(2):
                        mt = 2 * mu + jj
                        nc.tensor.matmul(
                            oh[:, 0:65],
                            ex_tiles[mu][ab][:, jj, j * P:(j + 1) * P],
                            vha[:, mt, h, :],
                            start=(mu == 0 and jj == 0),
                            stop=(mu == 7 and jj == 1),
                        )
                rr = nm.tile([P, 1], F32, tag="rr", name="rr")
                nc.vector._custom_dve(
                    RECIPROCAL_APPROX_FAST, out=rr[:], in0=oh[:, 64:65],
                    s0=_rc["s0"], s1=_rc["s1"], imm2=_rc["imm2"],
                )
                if ab == 0:
                    _mh2[j] = mh2_pool.tile([P, 2, DH], BF16, tag=f"mh2_{j}",
                                            name=f"mh2_{j}")
                mh2 = _mh2[j]
                nc.vector.tensor_scalar(
                    mh2[:, ab, :], oh[:, 0:64], rr[:, 0:1], None, MULT,
                )
                if ab == 1:
                    mtp = ps_wk.tile([P, 512], F32, tag="pj",
                                     name="pj").bitcast(BF16)[:, 0:P]
                    nc.tensor.transpose(
                        mtp, mh2.rearrange("p a b -> p (a b)"), identb[:])
                    nc.vector.tensor_copy(
                        mhT[nb][hp][:, j * P:(j + 1) * P], mtp)

            def emit_out_group(nt):
                """Output projection for global n-tile nt, PSUM -> HBM."""
                _mark("out_proj")
                nb, jl = nt // 4, nt % 4
                po = ps_wk.tile([P, DO], F32, tag="pj", name="pj")
                for hp in range(4):
                    nc.tensor.matmul(
                        po[:], mhT[nb][hp][:, jl * P:(jl + 1) * P],
                        wp_sb[:, hp, :],
                        start=(hp == 0), stop=False, skip_group_check=True,
                    )
                nc.tensor.matmul(
                    po[:], ones[0:1, 0:P], pb[:],
                    start=False, stop=True, skip_group_check=True,
                )
                ot = nm.tile([P, DO], F32, tag="ot", name="ot")
                nc.vector.tensor_copy(ot[:], po[:])
                (nc.gpsimd if nt % 2 else nc.sync).dma_start(
                    d_out[nt * P:(nt + 1) * P, :], ot[:])

            # === attention windows ===
            prev_ex = None
            for hp in range(4):
                for nb in range(2):
                    w = hp * 2 + nb
                    ex_tiles = [[None, None] for _ in range(8)]
                    for mu in range(8):
                        _mark(f"scores_w{w}")
                        for ab in range(2):
                            base = ab * 64
                            sc = ps_sc.tile([P, 1024], F32, tag="sc", name="sc")
                            for jj in range(2):
                                mt = 2 * mu + jj
                                nc.tensor.matmul(
                                    sc[:, jj * 512:(jj + 1) * 512],
                                    kTf[hp][base:base + 64, mt * P:(mt + 1) * P],
                                    qTf[hp][base:base + 64,
                                            nb * 512:(nb + 1) * 512],
                                    start=True, stop=True,
                                    tile_position=(base, 0),
                                )
                            ex = ex_pool.tile([P, 2, 512], BF16, tag="ex",
                                              name="ex")
                            nc.scalar.activation(
                                ex.rearrange("p a b -> p (a b)"), sc[:], EXP)
                            ex_tiles[mu][ab] = ex
                        # interleaved PE filler work
                        if w == 0:
                            emit_v_tile(2 * mu)
                            emit_v_tile(2 * mu + 1)
                            if mu % 2 == 1:
                                proj_k(1, mu // 2)
                        elif w == 1:
                            if mu % 4 == 1:
                                proj_q(1, mu // 4)
                        elif w in (2, 3):
                            ht = w
                            if mu % 2 == 0:
                                proj_k(ht, mu // 2)
                            elif mu % 4 == 1:
                                proj_q(ht, mu // 4)
                        # oh of the previous window
                        if w in (1, 2, 3, 4, 5, 6):
                            oh_group(w - 1, mu, prev_ex)
                        elif w == 7:
                            if mu < 4:
                                oh_group(6, 2 * mu, prev_ex)
                                oh_group(6, 2 * mu + 1, prev_ex)
                            else:
                                emit_out_group(mu - 4)
                    prev_ex = ex_tiles

            # === tail: window 7 oh + nb1 output projections ===
            for g in range(8):
                oh_group(7, g, prev_ex)
            for nt in range(4, 8):
                emit_out_group(nt)

    nc.compile()
    return nc


def kernel(query, key, value, query_kernel, key_kernel, value_kernel,
           projection_kernel, q_bias, k_bias, v_bias, projection_bias):
    query = np.ascontiguousarray(np.asarray(query, dtype=np.float32))
    key = np.ascontiguousarray(np.asarray(key, dtype=np.float32))
    value = np.ascontiguousarray(np.asarray(value, dtype=np.float32))
    scale = np.float32(1.0 / 8.0)  # 1/sqrt(DH)

    wq = np.ascontiguousarray(
        (np.asarray(query_kernel, np.float32) * scale).transpose(1, 0, 2).reshape(DM, HDH))
    wk = np.ascontiguousarray(
        np.asarray(key_kernel, np.float32).transpose(1, 0, 2).reshape(DM, HDH))
    wv = np.ascontiguousarray(
        np.asarray(value_kernel, np.float32).transpose(1, 0, 2).reshape(DM, HDH))
    wp = np.ascontiguousarray(
        np.asarray(projection_kernel, np.float32).reshape(HDH, DO)
    ).astype(ml_dtypes.bfloat16)
    qb = np.ascontiguousarray(
        (np.asarray(q_bias, np.float32) * scale).reshape(HDH).reshape(4, P).T)
    kb = np.ascontiguousarray(np.asarray(k_bias, np.float32).reshape(HDH).reshape(4, P).T)
    vbrow = np.ascontiguousarray(np.asarray(v_bias, np.float32).reshape(1, HDH))
    pb = np.ascontiguousarray(np.asarray(projection_bias, np.float32).reshape(1, DO))
    identb = np.eye(P, dtype=ml_dtypes.bfloat16)
    ident = np.eye(P, dtype=np.float32)
    ones = np.ones((P, P), dtype=np.float32)

    if "nc" not in _CACHED:
        _CACHED["nc"] = _build()
    nc = _CACHED["nc"]

    shared = dict(wq=wq, wk=wk, wv=wv, wp=wp, qb=qb, kb=kb, vbrow=vbrow, pb=pb,
                  identb=identb, ident=ident, ones=ones)
    in_maps = []
    for c in range(8):
        b, half = c // 2, c % 2
        in_maps.append(dict(
            q=np.ascontiguousarray(query[b, half * NB:(half + 1) * NB, :]),
            k=key[b], v=value[b], **shared))

    trace = os.environ.get("KERNEL_TRACE", "0") == "1"
    try:
        res = run_bass_kernel_spmd(nc, in_maps, core_ids=list(range(8)), trace=trace)
    except ModuleNotFoundError:
        res = run_bass_kernel_spmd(nc, in_maps, core_ids=list(range(8)), trace=False)
    global LAST_EXEC_NS
    LAST_EXEC_NS = res.exec_time_ns
    if trace and res.exec_time_ns is not None:
        print(f"HW exec time: {res.exec_time_ns} ns")
        if res.instructions_and_trace is not None:
            print(f"trace: {res.instructions_and_trace[1]}")

    B = query.shape[0]
    out = np.empty((B, 2 * NB, DO), dtype=np.float32)
    for c in range(8):
        b, half = c // 2, c % 2
        out[b, half * NB:(half + 1) * NB, :] = res.results[c]["out"]
    return out


# revision 14
# speedup vs baseline: 1.0533x; 1.0533x over previous
"""Multi-head attention Trainium2 Bass kernel.

Problem: B=4, N=M=2048, DM=512, H=8, DH=64, DO=512, fp32.
Sharding: 8 cores = (batch b, row-half) -- each core computes full attention
for 1024 query rows of one batch. No collectives.

Per-core dataflow (v2 -- oh flipped to [n, 65], bf16 attention operands):
  - PE-transpose Q,K,V 128x128 blocks (bf16 identity -> 1 cyc/row);
    transposed K/Q staging persists so per-head projections can be
    interleaved into later attention windows.
  - kTf/qTf [hdh, m|n] bf16 (bias + 1/sqrt(dh) folded host-side)
  - vha [m, h, 65] bf16 = [Vh + vb | 1]  (v-bias exact since sum(attn)=1)
  - scoresT[m, n] = kh @ qhT per head pair (tile_position row packing)
  - exp on ScalarE (PSUM fp32 -> SBUF bf16)
  - oh[n, 65] = ex^T(stationary) @ vha(moving, F=65); col 64 = denominator
  - normalize on DVE: per-partition reciprocal + multiply -> mh2 bf16
  - PE-transpose mh2 -> mhT [hdh, n] bf16
  - out[n, do] = sum_hp mhT_hp^T @ wp_hp + bias (ones-row matmul), PSUM->HBM
Loop nest: hp (head pair) outer, nb (n-half) inner; window w = hp*2+nb.
oh of window w-1 (+normalize+transpose) interleaves into window w's
scores/exp; V projection fills window 0; kTf/qTf head-pair projections fill
windows 1-3; output projections of nb0 fill window 7; nb1 outputs tail.
"""
import os
import sys

sys.path.insert(0, "/opt/trn_rl_repo")

import numpy as np
import ml_dtypes

import concourse.bass as bass
import concourse.mybir as mybir
import concourse.tile as tile
from concourse import bacc
from concourse.bass_utils import run_bass_kernel_spmd

F32 = mybir.dt.float32
F32R = mybir.dt.float32r
BF16 = mybir.dt.bfloat16
EXP = mybir.ActivationFunctionType.Exp
ADD = mybir.AluOpType.add
MULT = mybir.AluOpType.mult

P = 128
DM = 512
HDH = 512
DH = 64
H = 8
NB = 1024     # query rows per core
M = 2048      # kv rows
DO = 512
N_MT = M // P
N_QT = NB // P

_CACHED = {}
LAST_EXEC_NS = None
_SECTION = None  # optional trace-attribution hook: list whose [0] is set


def _mark(s):
    if _SECTION is not None:
        _SECTION[0] = s


def _build():
    nc = bacc.Bacc("TRN2", target_bir_lowering=False, debug=False)

    d_q = nc.declare_dram_parameter("q", [NB, DM], F32, isOutput=False)
    d_k = nc.declare_dram_parameter("k", [M, DM], F32, isOutput=False)
    d_v = nc.declare_dram_parameter("v", [M, DM], F32, isOutput=False)
    d_wq = nc.declare_dram_parameter("wq", [DM, HDH], F32R, isOutput=False)
    d_wk = nc.declare_dram_parameter("wk", [DM, HDH], F32R, isOutput=False)
    d_wv = nc.declare_dram_parameter("wv", [DM, HDH], F32R, isOutput=False)
    d_wp = nc.declare_dram_parameter("wp", [HDH, DO], BF16, isOutput=False)
    d_qb = nc.declare_dram_parameter("qb", [P, 4], F32, isOutput=False)
    d_kb = nc.declare_dram_parameter("kb", [P, 4], F32, isOutput=False)
    d_vbrow = nc.declare_dram_parameter("vbrow", [1, HDH], F32R, isOutput=False)
    d_pb = nc.declare_dram_parameter("pb", [1, DO], F32R, isOutput=False)
    d_idb = nc.declare_dram_parameter("identb", [P, P], BF16, isOutput=False)
    d_id = nc.declare_dram_parameter("ident", [P, P], F32R, isOutput=False)
    d_ones = nc.declare_dram_parameter("ones", [P, P], F32R, isOutput=False)
    d_out = nc.declare_dram_parameter("out", [NB, DO], F32, isOutput=True)

    with tile.TileContext(nc) as tc:
        from contextlib import ExitStack
        with ExitStack() as ctx:
            persist = ctx.enter_context(tc.tile_pool(name="persist", bufs=1))
            raw = ctx.enter_context(tc.tile_pool(name="raw", bufs=5))
            vtt_pool = ctx.enter_context(tc.tile_pool(name="vtt", bufs=3))
            ex_pool = ctx.enter_context(tc.tile_pool(name="expp", bufs=20))
            nm = ctx.enter_context(tc.tile_pool(name="nm", bufs=4))
            mh2_pool = ctx.enter_context(tc.tile_pool(name="mh2", bufs=3))
            ps_sc = ctx.enter_context(tc.tile_pool(name="ps_sc", bufs=3, space="PSUM"))
            ps_wk = ctx.enter_context(tc.tile_pool(name="ps_wk", bufs=2, space="PSUM"))

            # --- constants (first DMAs out) ---
            identb = persist.tile([P, P], BF16, tag="identb", name="identb")
            nc.sync.dma_start(identb[:], d_idb[:])
            ident = persist.tile([P, P], F32R, tag="ident", name="ident")
            nc.sync.dma_start(ident[:], d_id[:])
            qb = persist.tile([P, 4], F32, tag="qb", name="qb")
            nc.sync.dma_start(qb[:], d_qb[:])
            kb = persist.tile([P, 4], F32, tag="kb", name="kb")
            nc.sync.dma_start(kb[:], d_kb[:])
            ones = persist.tile([P, P], F32R, tag="ones", name="ones")
            nc.sync.dma_start(ones[:], d_ones[:])

            # --- persistent tensors ---
            kTf = [persist.tile([P, M], BF16, tag=f"kTf{i}", name=f"kTf{i}")
                   for i in range(4)]
            qTf = [persist.tile([P, NB], BF16, tag=f"qTf{i}", name=f"qTf{i}")
                   for i in range(4)]
            ktsK = [persist.tile([P, 4, 512], F32R, tag=f"ktsK{i}", name=f"ktsK{i}")
                    for i in range(4)]
            ktsQ = [persist.tile([P, 4, 512], F32R, tag=f"ktsQ{i}", name=f"ktsQ{i}")
                    for i in range(2)]
            vha = persist.tile([P, N_MT, H, 65], BF16, tag="vha", name="vha")
            mhT = [[persist.tile([P, 512], BF16, tag=f"mhT{nb}_{hp}",
                                 name=f"mhT{nb}_{hp}")
                    for hp in range(4)] for nb in range(2)]
            vbb = persist.tile([P, H, DH], BF16, tag="vbb", name="vbb")
            pb = persist.tile([1, DO], F32R, tag="pb", name="pb")
            vbrow = persist.tile([1, HDH], F32R, tag="vbrow", name="vbrow")
            wk_sb = [persist.tile([P, HDH], F32R, tag=f"wk{d}", name=f"wk{d}")
                     for d in range(4)]
            wq_sb = [persist.tile([P, HDH], F32R, tag=f"wq{d}", name=f"wq{d}")
                     for d in range(4)]
            wv_sb = [persist.tile([P, HDH], F32R, tag=f"wv{d}", name=f"wv{d}")
                     for d in range(4)]
            wp_sb = persist.tile([P, 4, DO], BF16, tag="wp", name="wp")

            def load_tile(d_src, t, eng=None):
                rn = raw.tile([P, DM], F32R, tag="araw", name="araw")
                (eng or nc.sync).dma_start(
                    rn[:], d_src[t * P:(t + 1) * P, :].bitcast(F32R))
                return rn

            def transpose_tiles(rns, ts):
                """Transpose 4 row-tiles into ts [P, 4, 512]
                ([dm-chunk, dc, row]). Copies on ScalarE (idle outside the
                attention windows)."""
                _mark("in_transpose")
                for j in range(4):
                    rn = rns[j]
                    pst = ps_wk.tile([P, DM], F32R, tag="pj", name="pj")
                    for dc in range(4):
                        nc.tensor.transpose(
                            pst[:, dc * P:(dc + 1) * P],
                            rn[:, dc * P:(dc + 1) * P], ident[:],
                        )
                    nc.scalar.copy(
                        ts[:, :, j * P:(j + 1) * P],
                        pst.rearrange("p (a b) -> p a b", a=4),
                    )

            def proj_k(ht, ms):
                """kTf[ht][:, ms*512:(ms+1)*512] from ktsK[ms]."""
                _mark(f"proj_k")
                pp = ps_sc.tile([P, 1024], F32, tag="sc", name="sc")
                for dc in range(4):
                    nc.tensor.matmul(
                        pp[:, 0:512], wk_sb[dc][:, ht * P:(ht + 1) * P],
                        ktsK[ms][:, dc, :], start=(dc == 0), stop=(dc == 3),
                    )
                nc.vector.tensor_scalar(
                    kTf[ht][:, ms * 512:(ms + 1) * 512],
                    pp[:, 0:512], kb[:, ht:ht + 1], None, ADD,
                )

            def proj_q(ht, ns):
                _mark(f"proj_q")
                pp = ps_sc.tile([P, 1024], F32, tag="sc", name="sc")
                for dc in range(4):
                    nc.tensor.matmul(
                        pp[:, 0:512], wq_sb[dc][:, ht * P:(ht + 1) * P],
                        ktsQ[ns][:, dc, :], start=(dc == 0), stop=(dc == 3),
                    )
                nc.vector.tensor_scalar(
                    qTf[ht][:, ns * 512:(ns + 1) * 512],
                    pp[:, 0:512], qb[:, ht:ht + 1], None, ADD,
                )

            # === lead-in: K transposes + kTf[0]; Q transposes + qTf[0] ===
            rns_k0 = [load_tile(d_k, j) for j in range(4)]
            for dcc in range(4):
                nc.sync.dma_start(wk_sb[dcc][:], d_wk[dcc * P:(dcc + 1) * P, :])
            stages = {0: rns_k0}
            for ms in range(1, 4):
                stages[ms] = [
                    load_tile(d_k, ms * 4 + j,
                              eng=nc.gpsimd if ms >= 2 else nc.sync)
                    for j in range(4)]
            for ms in range(4):
                transpose_tiles(stages[ms], ktsK[ms])
                proj_k(0, ms)
            for dcc in range(4):
                nc.gpsimd.dma_start(wq_sb[dcc][:], d_wq[dcc * P:(dcc + 1) * P, :])
            stq = [[load_tile(d_q, j) for j in range(4)],
                   [load_tile(d_q, 4 + j, eng=nc.gpsimd) for j in range(4)]]
            for ns in range(2):
                transpose_tiles(stq[ns], ktsQ[ns])
                proj_q(0, ns)
            for dcc in range(4):
                nc.gpsimd.dma_start(wv_sb[dcc][:], d_wv[dcc * P:(dcc + 1) * P, :])
            nc.gpsimd.dma_start(vbrow[:], d_vbrow[:])
            nc.gpsimd.dma_start(pb[:], d_pb[:])
            for a in range(4):
                nc.gpsimd.dma_start(wp_sb[:, a, :], d_wp[a * P:(a + 1) * P, :])
            # vbb = ones-col x vbrow: v-bias broadcast over m partitions
            bb = ps_wk.tile([P, DM], F32, tag="pj", name="pj")
            nc.tensor.matmul(bb[:], ones[0:1, 0:P], vbrow[:],
                             start=True, stop=True)
            nc.vector.tensor_copy(vbb.rearrange("p a b -> p (a b)"), bb[:])
            # ones column of vha
            nc.vector.tensor_copy(
                vha[:, :, :, 64:65],
                ones[:, 0:N_MT * H].bitcast(F32).rearrange(
                    "p (a h) -> p a h", a=N_MT)[:, :, :, None],
            )

            def emit_v_tile(mt):
                """Transpose + project one V m-tile into vha (+v-bias)."""
                _mark("v_tile")
                vn = load_tile(d_v, mt, eng=nc.gpsimd if mt % 2 else nc.sync)[:]
                pst = ps_wk.tile([P, DM], F32R, tag="pj", name="pj")
                for dc in range(4):
                    nc.tensor.transpose(
                        pst[:, dc * P:(dc + 1) * P], vn[:, dc * P:(dc + 1) * P],
                        ident[:],
                    )
                vtt = vtt_pool.tile([P, 4, P], F32R, tag="vtt", name="vtt")
                nc.vector.tensor_copy(vtt[:], pst.rearrange("p (a b) -> p a b", a=4))
                pp = ps_wk.tile([P, DM], F32, tag="pj", name="pj")
                for dc in range(4):
                    nc.tensor.matmul(
                        pp[:], vtt[:, dc, :], wv_sb[dc][:],
                        start=(dc == 0), stop=(dc == 3),
                    )
                nc.vector.tensor_tensor(
                    vha[:, mt, :, 0:64],
                    pp.rearrange("p (h c) -> p h c", h=H), vbb[:], ADD,
                )

            # recip consts
            from concourse.dve_ops import (
                RECIP_APPROX_FAST_CONSTS, RECIPROCAL_APPROX_FAST)
            _rc = RECIP_APPROX_FAST_CONSTS
            _mh2 = {}

            def oh_group(w, g, ex_tiles):
                """One oh accumulation group of window w: g = ab*4 + j.
                Accumulates oh[n-block j, 65] over all 16 m-tiles, then
                normalizes into mh2; emits the mh transpose after ab==1."""
                hp, nb = w // 2, w % 2
                ab, j = g // 4, g % 4
                _mark(f"oh_w{w}")
                h = 2 * hp + ab
                oh = ps_wk.tile([P, 512], F32, tag="pj", name="pj")
                for mu in range(8):
                    for jj in range(2):
                        mt = 2 * mu + jj
                        nc.tensor.matmul(
                            oh[:, 0:65],
                            ex_tiles[mu][ab][:, jj, j * P:(j + 1) * P],
                            vha[:, mt, h, :],
                            start=(mu == 0 and jj == 0),
                            stop=(mu == 7 and jj == 1),
                        )
                rr = nm.tile([P, 1], F32, tag="rr", name="rr")
                nc.vector._custom_dve(
                    RECIPROCAL_APPROX_FAST, out=rr[:], in0=oh[:, 64:65],
                    s0=_rc["s0"], s1=_rc["s1"], imm2=_rc["imm2"],
                )
                if ab == 0:
                    _mh2[j] = mh2_pool.tile([P, 2, DH], BF16, tag=f"mh2_{j}",
                                            name=f"mh2_{j}")
                mh2 = _mh2[j]
                nc.vector.tensor_scalar(
                    mh2[:, ab, :], oh[:, 0:64], rr[:, 0:1], None, MULT,
                )
                if ab == 1:
                    mtp = ps_wk.tile([P, 512], F32, tag="pj",
                                     name="pj").bitcast(BF16)[:, 0:P]
                    nc.tensor.transpose(
                        mtp, mh2.rearrange("p a b -> p (a b)"), identb[:])
                    nc.vector.tensor_copy(
                        mhT[nb][hp][:, j * P:(j + 1) * P], mtp)

            def emit_out_group(nt):
                """Output projection for global n-tile nt, PSUM -> HBM."""
                _mark("out_proj")
                nb, jl = nt // 4, nt % 4
                po = ps_wk.tile([P, DO], F32, tag="pj", name="pj")
                for hp in range(4):
                    nc.tensor.matmul(
                        po[:], mhT[nb][hp][:, jl * P:(jl + 1) * P],
                        wp_sb[:, hp, :],
                        start=(hp == 0), stop=False, skip_group_check=True,
                    )
                nc.tensor.matmul(
                    po[:], ones[0:1, 0:P], pb[:],
                    start=False, stop=True, skip_group_check=True,
                )
                ot = nm.tile([P, DO], F32, tag="ot", name="ot")
                nc.vector.tensor_copy(ot[:], po[:])
                (nc.gpsimd if nt % 2 else nc.sync).dma_start(
                    d_out[nt * P:(nt + 1) * P, :], ot[:])

            # === attention windows ===
            prev_ex = None
            for hp in range(4):
                for nb in range(2):
                    w = hp * 2 + nb
                    ex_tiles = [[None, None] for _ in range(8)]
                    ab_order = ([(mu, ab) for mu in range(8) for ab in range(2)]
                                if w < 7 else
                                [(mu, ab) for ab in range(2) for mu in range(8)])
                    for step, (mu, ab) in enumerate(ab_order):
                        _mark(f"scores_w{w}")
                        base = ab * 64
                        sc = ps_sc.tile([P, 1024], F32, tag="sc", name="sc")
                        for jj in range(2):
                            mt = 2 * mu + jj
                            nc.tensor.matmul(
                                sc[:, jj * 512:(jj + 1) * 512],
                                kTf[hp][base:base + 64, mt * P:(mt + 1) * P],
                                qTf[hp][base:base + 64,
                                        nb * 512:(nb + 1) * 512],
                                start=True, stop=True,
                                tile_position=(base, 0),
                            )
                        ex = ex_pool.tile([P, 2, 512], BF16, tag="ex",
                                          name="ex")
                        nc.scalar.activation(
                            ex.rearrange("p a b -> p (a b)"), sc[:], EXP)
                        ex_tiles[mu][ab] = ex
                        # interleaved PE filler work, one unit per step
                        if w == 0:
                            emit_v_tile(step)
                            if step % 4 == 3:
                                proj_k(1, step // 4)
                        elif w == 1:
                            if step % 8 == 1:
                                proj_q(1, step // 8)
                        elif w in (2, 3):
                            ht = w
                            if step % 4 == 0:
                                proj_k(ht, step // 4)
                            elif step % 8 == 1:
                                proj_q(ht, step // 8)
                        # oh of the previous window
                        if w in (1, 2, 3, 4, 5, 6):
                            if step % 2 == 1:
                                oh_group(w - 1, step // 2, prev_ex)
                        elif w == 7:
                            # ab0 phase (steps 0-7): window-6 oh groups
                            # ab1 phase (steps 8-15): nb0 outs + w7 ab0 oh
                            if step < 8:
                                oh_group(6, step, prev_ex)
                            elif step < 12:
                                emit_out_group(step - 8)
                            else:
                                oh_group(7, step - 12, ex_tiles)
                    prev_ex = ex_tiles

            # === tail: window 7 ab1 oh groups + nb1 output projections ===
            for j in range(4):
                oh_group(7, 4 + j, prev_ex)
                emit_out_group(4 + j)

    nc.compile()
    return nc


def kernel(query, key, value, query_kernel, key_kernel, value_kernel,
           projection_kernel, q_bias, k_bias, v_bias, projection_bias):
    query = np.ascontiguousarray(np.asarray(query, dtype=np.float32))
    key = np.ascontiguousarray(np.asarray(key, dtype=np.float32))
    value = np.ascontiguousarray(np.asarray(value, dtype=np.float32))
    scale = np.float32(1.0 / 8.0)  # 1/sqrt(DH)

    wq = np.ascontiguousarray(
        (np.asarray(query_kernel, np.float32) * scale).transpose(1, 0, 2).reshape(DM, HDH))
    wk = np.ascontiguousarray(
        np.asarray(key_kernel, np.float32).transpose(1, 0, 2).reshape(DM, HDH))
    wv = np.ascontiguousarray(
        np.asarray(value_kernel, np.float32).transpose(1, 0, 2).reshape(DM, HDH))
    wp = np.ascontiguousarray(
        np.asarray(projection_kernel, np.float32).reshape(HDH, DO)
    ).astype(ml_dtypes.bfloat16)
    qb = np.ascontiguousarray(
        (np.asarray(q_bias, np.float32) * scale).reshape(HDH).reshape(4, P).T)
    kb = np.ascontiguousarray(np.asarray(k_bias, np.float32).reshape(HDH).reshape(4, P).T)
    vbrow = np.ascontiguousarray(np.asarray(v_bias, np.float32).reshape(1, HDH))
    pb = np.ascontiguousarray(np.asarray(projection_bias, np.float32).reshape(1, DO))
    identb = np.eye(P, dtype=ml_dtypes.bfloat16)
    ident = np.eye(P, dtype=np.float32)
    ones = np.ones((P, P), dtype=np.float32)

    if "nc" not in _CACHED:
        _CACHED["nc"] = _build()
    nc = _CACHED["nc"]

    shared = dict(wq=wq, wk=wk, wv=wv, wp=wp, qb=qb, kb=kb, vbrow=vbrow, pb=pb,
                  identb=identb, ident=ident, ones=ones)
    in_maps = []
    for c in range(8):
        b, half = c // 2, c % 2
        in_maps.append(dict(
            q=np.ascontiguousarray(query[b, half * NB:(half + 1) * NB, :]),
            k=key[b], v=value[b], **shared))

    trace = os.environ.get("KERNEL_TRACE", "0") == "1"
    try:
        res = run_bass_kernel_spmd(nc, in_maps, core_ids=list(range(8)), trace=trace)
    except ModuleNotFoundError:
        res = run_bass_kernel_spmd(nc, in_maps, core_ids=list(range(8)), trace=False)
    global LAST_EXEC_NS
    LAST_EXEC_NS = res.exec_time_ns
    if trace and res.exec_time_ns is not None:
        print(f"HW exec time: {res.exec_time_ns} ns")
        if res.instructions_and_trace is not None:
            print(f"trace: {res.instructions_and_trace[1]}")

    B = query.shape[0]
    out = np.empty((B, 2 * NB, DO), dtype=np.float32)
    for c in range(8):
        b, half = c // 2, c % 2
        out[b, half * NB:(half + 1) * NB, :] = res.results[c]["out"]
    return out


# revision 17
# speedup vs baseline: 1.0652x; 1.0113x over previous
"""Multi-head attention Trainium2 Bass kernel.

Problem: B=4, N=M=2048, DM=512, H=8, DH=64, DO=512, fp32.
Sharding: 8 cores = (batch b, row-half) -- each core computes full attention
for 1024 query rows of one batch. No collectives.

Per-core dataflow (v2 -- oh flipped to [n, 65], bf16 attention operands):
  - PE-transpose Q,K,V 128x128 blocks (bf16 identity -> 1 cyc/row);
    transposed K/Q staging persists so per-head projections can be
    interleaved into later attention windows.
  - kTf/qTf [hdh, m|n] bf16 (bias + 1/sqrt(dh) folded host-side)
  - vha [m, h, 65] bf16 = [Vh + vb | 1]  (v-bias exact since sum(attn)=1)
  - scoresT[m, n] = kh @ qhT per head pair (tile_position row packing)
  - exp on ScalarE (PSUM fp32 -> SBUF bf16)
  - oh[n, 65] = ex^T(stationary) @ vha(moving, F=65); col 64 = denominator
  - normalize on DVE: per-partition reciprocal + multiply -> mh2 bf16
  - PE-transpose mh2 -> mhT [hdh, n] bf16
  - out[n, do] = sum_hp mhT_hp^T @ wp_hp + bias (ones-row matmul), PSUM->HBM
Loop nest: hp (head pair) outer, nb (n-half) inner; window w = hp*2+nb.
oh of window w-1 (+normalize+transpose) interleaves into window w's
scores/exp; V projection fills window 0; kTf/qTf head-pair projections fill
windows 1-3; output projections of nb0 fill window 7; nb1 outputs tail.
"""
import os
import sys

sys.path.insert(0, "/opt/trn_rl_repo")

import numpy as np
import ml_dtypes

import concourse.bass as bass
import concourse.mybir as mybir
import concourse.tile as tile
from concourse import bacc
from concourse.bass_utils import run_bass_kernel_spmd

F32 = mybir.dt.float32
F32R = mybir.dt.float32r
BF16 = mybir.dt.bfloat16
EXP = mybir.ActivationFunctionType.Exp
ADD = mybir.AluOpType.add
MULT = mybir.AluOpType.mult

P = 128
DM = 512
HDH = 512
DH = 64
H = 8
NB = 1024     # query rows per core
M = 2048      # kv rows
DO = 512
N_MT = M // P
N_QT = NB // P

_CACHED = {}
LAST_EXEC_NS = None
_SECTION = None  # optional trace-attribution hook: list whose [0] is set


def _mark(s):
    if _SECTION is not None:
        _SECTION[0] = s


def _build():
    nc = bacc.Bacc("TRN2", target_bir_lowering=False, debug=False)

    d_q = nc.declare_dram_parameter("q", [NB, DM], F32, isOutput=False)
    d_k = nc.declare_dram_parameter("k", [M, DM], F32, isOutput=False)
    d_v = nc.declare_dram_parameter("v", [M, DM], F32, isOutput=False)
    d_wq = nc.declare_dram_parameter("wq", [DM, HDH], F32R, isOutput=False)
    d_wk = nc.declare_dram_parameter("wk", [DM, HDH], F32R, isOutput=False)
    d_wv = nc.declare_dram_parameter("wv", [DM, HDH], F32R, isOutput=False)
    d_wp = nc.declare_dram_parameter("wp", [HDH, DO], BF16, isOutput=False)
    d_qb = nc.declare_dram_parameter("qb", [P, 4], F32, isOutput=False)
    d_kb = nc.declare_dram_parameter("kb", [P, 4], F32, isOutput=False)
    d_vbrow = nc.declare_dram_parameter("vbrow", [1, HDH], F32R, isOutput=False)
    d_pb = nc.declare_dram_parameter("pb", [1, DO], F32R, isOutput=False)
    d_idb = nc.declare_dram_parameter("identb", [P, P], BF16, isOutput=False)
    d_id = nc.declare_dram_parameter("ident", [P, P], F32R, isOutput=False)
    d_ones = nc.declare_dram_parameter("ones", [P, P], F32R, isOutput=False)
    d_out = nc.declare_dram_parameter("out", [NB, DO], F32, isOutput=True)

    with tile.TileContext(nc) as tc:
        from contextlib import ExitStack
        with ExitStack() as ctx:
            persist = ctx.enter_context(tc.tile_pool(name="persist", bufs=1))
            raw = ctx.enter_context(tc.tile_pool(name="raw", bufs=5))
            vtt_pool = ctx.enter_context(tc.tile_pool(name="vtt", bufs=3))
            ex_pool = ctx.enter_context(tc.tile_pool(name="expp", bufs=20))
            nm = ctx.enter_context(tc.tile_pool(name="nm", bufs=4))
            mh2_pool = ctx.enter_context(tc.tile_pool(name="mh2", bufs=3))
            ps_sc = ctx.enter_context(tc.tile_pool(name="ps_sc", bufs=3, space="PSUM"))
            ps_wk = ctx.enter_context(tc.tile_pool(name="ps_wk", bufs=2, space="PSUM"))

            # --- constants (first DMAs out) ---
            identb = persist.tile([P, P], BF16, tag="identb", name="identb")
            nc.sync.dma_start(identb[:], d_idb[:])
            ident = persist.tile([P, P], F32R, tag="ident", name="ident")
            nc.sync.dma_start(ident[:], d_id[:])
            qb = persist.tile([P, 4], F32, tag="qb", name="qb")
            nc.sync.dma_start(qb[:], d_qb[:])
            kb = persist.tile([P, 4], F32, tag="kb", name="kb")
            nc.sync.dma_start(kb[:], d_kb[:])
            ones = persist.tile([P, P], F32R, tag="ones", name="ones")
            nc.sync.dma_start(ones[:], d_ones[:])

            # --- persistent tensors ---
            kTf = [persist.tile([P, M], BF16, tag=f"kTf{i}", name=f"kTf{i}")
                   for i in range(4)]
            qTf = [persist.tile([P, NB], BF16, tag=f"qTf{i}", name=f"qTf{i}")
                   for i in range(4)]
            ktsK = [persist.tile([P, 4, 512], F32R, tag=f"ktsK{i}", name=f"ktsK{i}")
                    for i in range(4)]
            ktsQ = [persist.tile([P, 4, 512], F32R, tag=f"ktsQ{i}", name=f"ktsQ{i}")
                    for i in range(2)]
            vha = persist.tile([P, N_MT, H, 65], BF16, tag="vha", name="vha")
            mhT = [[persist.tile([P, 512], BF16, tag=f"mhT{nb}_{hp}",
                                 name=f"mhT{nb}_{hp}")
                    for hp in range(4)] for nb in range(2)]
            vbb = persist.tile([P, H, DH], BF16, tag="vbb", name="vbb")
            pb = persist.tile([1, DO], F32R, tag="pb", name="pb")
            vbrow = persist.tile([1, HDH], F32R, tag="vbrow", name="vbrow")
            wk_sb = [persist.tile([P, HDH], F32R, tag=f"wk{d}", name=f"wk{d}")
                     for d in range(4)]
            wq_sb = [persist.tile([P, HDH], F32R, tag=f"wq{d}", name=f"wq{d}")
                     for d in range(4)]
            wv_sb = [persist.tile([P, HDH], F32R, tag=f"wv{d}", name=f"wv{d}")
                     for d in range(4)]
            wp_sb = persist.tile([P, 4, DO], BF16, tag="wp", name="wp")

            def load_tile(d_src, t, eng=None):
                rn = raw.tile([P, DM], F32R, tag="araw", name="araw")
                (eng or nc.sync).dma_start(
                    rn[:], d_src[t * P:(t + 1) * P, :].bitcast(F32R))
                return rn

            def transpose_tiles(rns, ts):
                """Transpose 4 row-tiles into ts [P, 4, 512]
                ([dm-chunk, dc, row]). Copies on ScalarE (idle outside the
                attention windows)."""
                _mark("in_transpose")
                for j in range(4):
                    rn = rns[j]
                    pst = ps_wk.tile([P, DM], F32R, tag="pj", name="pj")
                    for dc in range(4):
                        nc.tensor.transpose(
                            pst[:, dc * P:(dc + 1) * P],
                            rn[:, dc * P:(dc + 1) * P], ident[:],
                        )
                    eng = nc.scalar.copy if j % 2 == 0 else nc.vector.tensor_copy
                    eng(
                        ts[:, :, j * P:(j + 1) * P],
                        pst.rearrange("p (a b) -> p a b", a=4),
                    )

            _vtiles = {}

            def load_v_tile(mt):
                _vtiles[mt] = load_tile(d_v, mt,
                                        eng=nc.gpsimd if mt % 2 else nc.sync)

            def proj_k(ht, ms):
                """kTf[ht][:, ms*512:(ms+1)*512] from ktsK[ms]."""
                _mark(f"proj_k")
                pp = ps_sc.tile([P, 1024], F32, tag="sc", name="sc")
                for dc in range(4):
                    nc.tensor.matmul(
                        pp[:, 0:512], wk_sb[dc][:, ht * P:(ht + 1) * P],
                        ktsK[ms][:, dc, :], start=(dc == 0), stop=(dc == 3),
                    )
                nc.vector.tensor_scalar(
                    kTf[ht][:, ms * 512:(ms + 1) * 512],
                    pp[:, 0:512], kb[:, ht:ht + 1], None, ADD,
                )

            def proj_q(ht, ns):
                _mark(f"proj_q")
                pp = ps_sc.tile([P, 1024], F32, tag="sc", name="sc")
                for dc in range(4):
                    nc.tensor.matmul(
                        pp[:, 0:512], wq_sb[dc][:, ht * P:(ht + 1) * P],
                        ktsQ[ns][:, dc, :], start=(dc == 0), stop=(dc == 3),
                    )
                nc.vector.tensor_scalar(
                    qTf[ht][:, ns * 512:(ns + 1) * 512],
                    pp[:, 0:512], qb[:, ht:ht + 1], None, ADD,
                )

            # === lead-in: K transposes + kTf[0]; Q transposes + qTf[0] ===
            rns_k0 = [load_tile(d_k, j) for j in range(4)]
            for dcc in range(4):
                nc.sync.dma_start(wk_sb[dcc][:], d_wk[dcc * P:(dcc + 1) * P, :])
            stages = {0: rns_k0}
            for ms in range(1, 4):
                stages[ms] = [
                    load_tile(d_k, ms * 4 + j,
                              eng=nc.gpsimd if ms >= 2 else nc.sync)
                    for j in range(4)]
            for ms in range(4):
                transpose_tiles(stages[ms], ktsK[ms])
                proj_k(0, ms)
            for dcc in range(4):
                nc.gpsimd.dma_start(wq_sb[dcc][:], d_wq[dcc * P:(dcc + 1) * P, :])
            stq0 = [load_tile(d_q, j) for j in range(4)]
            transpose_tiles(stq0, ktsQ[0])
            proj_q(0, 0)
            for j in range(3):
                load_v_tile(j)
            stq1 = [load_tile(d_q, 4 + j, eng=nc.gpsimd) for j in range(4)]
            for dcc in range(4):
                nc.gpsimd.dma_start(wv_sb[dcc][:], d_wv[dcc * P:(dcc + 1) * P, :])
            nc.gpsimd.dma_start(vbrow[:], d_vbrow[:])
            nc.gpsimd.dma_start(pb[:], d_pb[:])
            for a in range(4):
                nc.gpsimd.dma_start(wp_sb[:, a, :], d_wp[a * P:(a + 1) * P, :])
            # vbb = ones-col x vbrow: v-bias broadcast over m partitions
            bb = ps_wk.tile([P, DM], F32, tag="pj", name="pj")
            nc.tensor.matmul(bb[:], ones[0:1, 0:P], vbrow[:],
                             start=True, stop=True)
            nc.vector.tensor_copy(vbb.rearrange("p a b -> p (a b)"), bb[:])
            # ones column of vha
            nc.vector.tensor_copy(
                vha[:, :, :, 64:65],
                ones[:, 0:N_MT * H].bitcast(F32).rearrange(
                    "p (a h) -> p a h", a=N_MT)[:, :, :, None],
            )

            def emit_v_tile(mt):
                """Transpose + project one V m-tile into vha (+v-bias)."""
                _mark("v_tile")
                vn = _vtiles.pop(mt)[:]
                pst = ps_wk.tile([P, DM], F32R, tag="pj", name="pj")
                for dc in range(4):
                    nc.tensor.transpose(
                        pst[:, dc * P:(dc + 1) * P], vn[:, dc * P:(dc + 1) * P],
                        ident[:],
                    )
                vtt = vtt_pool.tile([P, 4, P], F32R, tag="vtt", name="vtt")
                nc.vector.tensor_copy(vtt[:], pst.rearrange("p (a b) -> p a b", a=4))
                pp = ps_wk.tile([P, DM], F32, tag="pj", name="pj")
                for dc in range(4):
                    nc.tensor.matmul(
                        pp[:], vtt[:, dc, :], wv_sb[dc][:],
                        start=(dc == 0), stop=(dc == 3),
                    )
                nc.vector.tensor_tensor(
                    vha[:, mt, :, 0:64],
                    pp.rearrange("p (h c) -> p h c", h=H), vbb[:], ADD,
                )

            # recip consts
            from concourse.dve_ops import (
                RECIP_APPROX_FAST_CONSTS, RECIPROCAL_APPROX_FAST)
            _rc = RECIP_APPROX_FAST_CONSTS
            _mh2 = {}

            def oh_group(w, g, ex_tiles, pool=None):
                """One oh accumulation group of window w: g = ab*4 + j.
                Accumulates oh[n-block j, 65] over all 16 m-tiles, then
                normalizes into mh2; emits the mh transpose after ab==1."""
                hp, nb = w // 2, w % 2
                ab, j = g // 4, g % 4
                _mark(f"oh_w{w}")
                h = 2 * hp + ab
                if pool is None:
                    oh = ps_wk.tile([P, 512], F32, tag="pj", name="pj")
                else:
                    oh = pool.tile([P, 1024], F32, tag="sc", name="sc")
                for mu in range(8):
                    for jj in range(2):
                        mt = 2 * mu + jj
                        nc.tensor.matmul(
                            oh[:, 0:65],
                            ex_tiles[mu][ab][:, jj, j * P:(j + 1) * P],
                            vha[:, mt, h, :],
                            start=(mu == 0 and jj == 0),
                            stop=(mu == 7 and jj == 1),
                        )
                rr = nm.tile([P, 1], F32, tag="rr", name="rr")
                nc.vector._custom_dve(
                    RECIPROCAL_APPROX_FAST, out=rr[:], in0=oh[:, 64:65],
                    s0=_rc["s0"], s1=_rc["s1"], imm2=_rc["imm2"],
                )
                if ab == 0:
                    _mh2[j] = mh2_pool.tile([P, 2, DH], BF16, tag=f"mh2_{j}",
                                            name=f"mh2_{j}")
                mh2 = _mh2[j]
                nc.vector.tensor_scalar(
                    mh2[:, ab, :], oh[:, 0:64], rr[:, 0:1], None, MULT,
                )
                if ab == 1:
                    mtp = ps_wk.tile([P, 512], F32, tag="pj",
                                     name="pj").bitcast(BF16)[:, 0:P]
                    nc.tensor.transpose(
                        mtp, mh2.rearrange("p a b -> p (a b)"), identb[:])
                    nc.vector.tensor_copy(
                        mhT[nb][hp][:, j * P:(j + 1) * P], mtp)

            def emit_out_group(nt):
                """Output projection for global n-tile nt, PSUM -> HBM."""
                _mark("out_proj")
                nb, jl = nt // 4, nt % 4
                po = ps_wk.tile([P, DO], F32, tag="pj", name="pj")
                for hp in range(4):
                    nc.tensor.matmul(
                        po[:], mhT[nb][hp][:, jl * P:(jl + 1) * P],
                        wp_sb[:, hp, :],
                        start=(hp == 0), stop=False, skip_group_check=True,
                    )
                nc.tensor.matmul(
                    po[:], ones[0:1, 0:P], pb[:],
                    start=False, stop=True, skip_group_check=True,
                )
                ot = nm.tile([P, DO], F32, tag="ot", name="ot")
                nc.vector.tensor_copy(ot[:], po[:])
                (nc.gpsimd if nt % 2 else nc.sync).dma_start(
                    d_out[nt * P:(nt + 1) * P, :], ot[:])

            # === attention windows ===
            prev_ex = None
            for hp in range(4):
                for nb in range(2):
                    w = hp * 2 + nb
                    ex_tiles = [[None, None] for _ in range(8)]
                    ab_order = ([(mu, ab) for mu in range(8) for ab in range(2)]
                                if w < 7 else
                                [(mu, ab) for ab in range(2) for mu in range(8)])
                    for step, (mu, ab) in enumerate(ab_order):
                        _mark(f"scores_w{w}")
                        base = ab * 64
                        sc = ps_sc.tile([P, 1024], F32, tag="sc", name="sc")
                        for jj in range(2):
                            mt = 2 * mu + jj
                            nc.tensor.matmul(
                                sc[:, jj * 512:(jj + 1) * 512],
                                kTf[hp][base:base + 64, mt * P:(mt + 1) * P],
                                qTf[hp][base:base + 64,
                                        nb * 512:(nb + 1) * 512],
                                start=True, stop=True,
                                tile_position=(base, 0),
                            )
                        ex = ex_pool.tile([P, 2, 512], BF16, tag="ex",
                                          name="ex")
                        nc.scalar.activation(
                            ex.rearrange("p a b -> p (a b)"), sc[:], EXP)
                        ex_tiles[mu][ab] = ex
                        # interleaved PE filler work, one unit per step
                        if w == 0:
                            if step + 3 < 16:
                                load_v_tile(step + 3)
                            emit_v_tile(step)
                            if step == 1:
                                transpose_tiles(stq1, ktsQ[1])
                            elif step == 3:
                                proj_q(0, 1)
                            elif step in (5, 9, 13):
                                proj_k(1, (step - 5) // 4)
                        elif w == 1:
                            if step == 3:
                                proj_k(1, 3)
                            elif step in (1, 9):
                                proj_q(1, step // 8)
                        elif w in (2, 3):
                            ht = w
                            if step % 4 == 0:
                                proj_k(ht, step // 4)
                            elif step % 8 == 1:
                                proj_q(ht, step // 8)
                        # oh of the previous window
                        if w in (1, 2, 3, 4, 5, 6):
                            if step % 2 == 1:
                                oh_group(w - 1, step // 2, prev_ex)
                        elif w == 7:
                            # ab0 phase (steps 0-7): window-6 oh groups
                            # ab1 phase (steps 8-15): nb0 outs + w7 ab0 oh
                            if step < 8:
                                oh_group(6, step, prev_ex)
                            elif step < 12:
                                emit_out_group(step - 8)
                            else:
                                oh_group(7, step - 12, ex_tiles)
                    prev_ex = ex_tiles

            # === tail: window 7 ab1 oh groups + nb1 output projections ===
            for j in range(4):
                oh_group(7, 4 + j, prev_ex, pool=ps_sc)
                emit_out_group(4 + j)

    nc.compile()
    return nc


def kernel(query, key, value, query_kernel, key_kernel, value_kernel,
           projection_kernel, q_bias, k_bias, v_bias, projection_bias):
    query = np.ascontiguousarray(np.asarray(query, dtype=np.float32))
    key = np.ascontiguousarray(np.asarray(key, dtype=np.float32))
    value = np.ascontiguousarray(np.asarray(value, dtype=np.float32))
    scale = np.float32(1.0 / 8.0)  # 1/sqrt(DH)

    wq = np.ascontiguousarray(
        (np.asarray(query_kernel, np.float32) * scale).transpose(1, 0, 2).reshape(DM, HDH))
    wk = np.ascontiguousarray(
        np.asarray(key_kernel, np.float32).transpose(1, 0, 2).reshape(DM, HDH))
    wv = np.ascontiguousarray(
        np.asarray(value_kernel, np.float32).transpose(1, 0, 2).reshape(DM, HDH))
    wp = np.ascontiguousarray(
        np.asarray(projection_kernel, np.float32).reshape(HDH, DO)
    ).astype(ml_dtypes.bfloat16)
    qb = np.ascontiguousarray(
        (np.asarray(q_bias, np.float32) * scale).reshape(HDH).reshape(4, P).T)
    kb = np.ascontiguousarray(np.asarray(k_bias, np.float32).reshape(HDH).reshape(4, P).T)
    vbrow = np.ascontiguousarray(np.asarray(v_bias, np.float32).reshape(1, HDH))
    pb = np.ascontiguousarray(np.asarray(projection_bias, np.float32).reshape(1, DO))
    identb = np.eye(P, dtype=ml_dtypes.bfloat16)
    ident = np.eye(P, dtype=np.float32)
    ones = np.ones((P, P), dtype=np.float32)

    if "nc" not in _CACHED:
        _CACHED["nc"] = _build()
    nc = _CACHED["nc"]

    shared = dict(wq=wq, wk=wk, wv=wv, wp=wp, qb=qb, kb=kb, vbrow=vbrow, pb=pb,
                  identb=identb, ident=ident, ones=ones)
    in_maps = []
    for c in range(8):
        b, half = c // 2, c % 2
        in_maps.append(dict(
            q=np.ascontiguousarray(query[b, half * NB:(half + 1) * NB, :]),
            k=key[b], v=value[b], **shared))

    trace = os.environ.get("KERNEL_TRACE", "0") == "1"
    try:
        res = run_bass_kernel_spmd(nc, in_maps, core_ids=list(range(8)), trace=trace)
    except ModuleNotFoundError:
        res = run_bass_kernel_spmd(nc, in_maps, core_ids=list(range(8)), trace=False)
    global LAST_EXEC_NS
    LAST_EXEC_NS = res.exec_time_ns
    if trace and res.exec_time_ns is not None:
        print(f"HW exec time: {res.exec_time_ns} ns")
        if res.instructions_and_trace is not None:
            print(f"trace: {res.instructions_and_trace[1]}")

    B = query.shape[0]
    out = np.empty((B, 2 * NB, DO), dtype=np.float32)
    for c in range(8):
        b, half = c // 2, c % 2
        out[b, half * NB:(half + 1) * NB, :] = res.results[c]["out"]
    return out
